# revision 18
# baseline (speedup 1.0000x reference)
"""Trainium2 Bass kernel for nn_BackwardTransformLayer (inverse wavelet step).

Math (polyphase form of the reference):
    g = flip(scaling_rec); g[1::2] *= -1
    out[i, 2u]   = sum_{j=0..3} g[2j]   * d[i, (u+j)   % M] + s[2j]   * a[i, (u+j)   % M]
    out[i, 2u+1] = sum_{j=0..3} g[2j+1] * d[i, (u+1+j) % M] + s[2j+1] * a[i, (u+1+j) % M]

i.e. two 4-tap circular FIRs along the free dim per output polyphase, summed
(16 MACs per input column).  Shifts are free (SBUF column-offset views).

Engine split (per core, 512 rows):
  - PE region (u in [0, PE_U)): taps as scaled-identity matmuls accumulating
    in PSUM.  fp32 matmul is 4 cyc/row on TRN2, so inputs are split host-side
    into fp16 hi + fp16 lo (same total bytes as fp32) and each tap runs as
    3 full-rate fp16 matmuls: c_hi*x_hi + c_hi*x_lo + c_lo*x_hi  (the dropped
    c_lo*x_lo term is ~2^-22 relative).  ScalarE drains PSUM into the output
    tile with a stride-2 write that interleaves even/odd.
  - DVE region (u in [PE_U, M)): fp32 scalar_tensor_tensor FMA chains; the
    last tap writes interleaved (stride-2) directly.

Sharding: embarrassingly parallel over rows; 512 rows per core x 8 cores.
"""

import numpy as np

P = 128                        # SBUF partitions
M = 8192                       # input columns
N_ROWS = 4096
N_CORES = 8
ROWS_PER_CORE = N_ROWS // N_CORES    # 512
OUT_M = 2 * M
HALO = 4                       # max shift reach (odd half uses j+1 <= 4)
NBLK = ROWS_PER_CORE // P      # 4 row blocks per core

PE_U = 3648                    # columns computed on PE (fp16 x3 path)
DVE_U = M - PE_U               # columns computed on DVE (fp32 path) = 4544
PE_PAN = 1824                  # PE panel width (2 panels)
DVE_PAN = 2272                 # DVE panel width (2 panels)
_CACHE = {}


def _tap_src_shift(t):
    """Tap table: t=0..7 feed the even polyphase, 8..15 the odd one."""
    src = 0 if (t % 8) < 4 else 1        # 0 -> details, 1 -> approximation
    shift = t % 4 + (1 if t >= 8 else 0)
    return src, shift


def _build(reps=1):
    import contextlib

    import concourse.bacc as bacc
    import concourse.mybir as mybir
    from concourse.tile import TileContext

    f32 = mybir.dt.float32
    f16 = mybir.dt.float16
    Alu = mybir.AluOpType

    nc = bacc.Bacc("TRN2", target_bir_lowering=False, debug=False)
    dh = nc.dram_tensor("dh", [ROWS_PER_CORE, PE_U + HALO], f16, kind="ExternalInput")
    dl = nc.dram_tensor("dl", [ROWS_PER_CORE, PE_U + HALO], f16, kind="ExternalInput")
    ah = nc.dram_tensor("ah", [ROWS_PER_CORE, PE_U + HALO], f16, kind="ExternalInput")
    al = nc.dram_tensor("al", [ROWS_PER_CORE, PE_U + HALO], f16, kind="ExternalInput")
    dv = nc.dram_tensor("dv", [ROWS_PER_CORE, DVE_U + HALO], f32, kind="ExternalInput")
    av = nc.dram_tensor("av", [ROWS_PER_CORE, DVE_U + HALO], f32, kind="ExternalInput")
    wh = nc.dram_tensor("wh", [P, 16 * P], f16, kind="ExternalInput")
    wl = nc.dram_tensor("wl", [P, 16 * P], f16, kind="ExternalInput")
    cc = nc.dram_tensor("cc", [P, 16], f32, kind="ExternalInput")
    o = nc.dram_tensor("o", [ROWS_PER_CORE, OUT_M], f32, kind="ExternalOutput")

    with TileContext(nc) as tc:
        with (
            tc.tile_pool(name="const", bufs=1) as const_pool,
            tc.tile_pool(name="pein", bufs=3) as pein_pool,
            tc.tile_pool(name="peout", bufs=2) as peout_pool,
            tc.tile_pool(name="dvein", bufs=2) as dvein_pool,
            tc.tile_pool(name="dveout", bufs=2) as dveout_pool,
            tc.tile_pool(name="acc", bufs=2) as acc_pool,
            tc.tile_pool(name="psum", bufs=8, space="PSUM") as psum_pool,
        ):
            wh_sb = const_pool.tile([P, 16 * P], f16)
            nc.sync.dma_start(out=wh_sb[:], in_=wh[:])
            c_sb = const_pool.tile([P, 16], f32)
            nc.scalar.dma_start(out=c_sb[:], in_=cc[:])
            # wl is first needed ~16 matmuls into the first PSUM group
            # (pass-major order) — defer its DMA behind the first panel's
            # hi tiles so it doesn't delay the first matmul.
            wl_sb = const_pool.tile([P, 16 * P], f16)
            wl_loaded = [False]

            rep_ctx = tc.For_i(0, reps, 1) if reps > 1 else contextlib.nullcontext()
            with rep_ctx:
                for blk in range(NBLK):
                    r0 = blk * P

                    # First block's leading panels are halved so the first
                    # compute op starts after less DMA (shorter pipeline fill);
                    # last block's trailing panels are halved so the final
                    # stores and tail drain start earlier.
                    if blk == 0:
                        pe_widths = [PE_PAN // 4, PE_PAN // 4, PE_PAN // 2, PE_PAN]
                        dve_widths = [DVE_PAN // 4, DVE_PAN // 4, DVE_PAN // 2, DVE_PAN]
                    elif blk == NBLK - 1:
                        pe_widths = [PE_PAN, PE_PAN // 2, PE_PAN // 2]
                        dve_widths = [DVE_PAN, DVE_PAN // 2, DVE_PAN // 2]
                    else:
                        pe_widths = [PE_PAN, PE_PAN]
                        dve_widths = [DVE_PAN, DVE_PAN]

                    # ---------------- PE region ----------------
                    p0 = 0
                    for pw in pe_widths:
                        tw = pw + HALO
                        dh_t = pein_pool.tile([P, tw], f16, tag="dh")
                        dl_t = pein_pool.tile([P, tw], f16, tag="dl")
                        ah_t = pein_pool.tile([P, tw], f16, tag="ah")
                        al_t = pein_pool.tile([P, tw], f16, tag="al")
                        for src_dram, t_sb in ((dh, dh_t), (ah, ah_t), (dl, dl_t), (al, al_t)):
                            nc.sync.dma_start(
                                out=t_sb[:], in_=src_dram[r0:r0 + P, p0:p0 + tw]
                            )
                        if not wl_loaded[0]:
                            nc.sync.dma_start(out=wl_sb[:], in_=wl[:])
                            wl_loaded[0] = True
                        hi = (dh_t, ah_t)
                        lo = (dl_t, al_t)

                        chunks = []
                        rem = pw
                        while rem > 0:
                            c = min(512, rem)
                            chunks.append(c)
                            rem -= c

                        o_t = peout_pool.tile([P, 2 * pw], f32, tag="peo")
                        coff = 0
                        for cw in chunks:
                            for half in (0, 1):
                                ps = psum_pool.tile([P, cw], f32, tag="ps")
                                taps = list(range(8 * half, 8 * half + 8))
                                n_mm = 3 * len(taps)
                                i_mm = 0
                                for w_sel, x_sel in ((wh_sb, hi), (wh_sb, lo), (wl_sb, hi)):
                                    for t in taps:
                                        src, j = _tap_src_shift(t)
                                        c0 = coff + j
                                        nc.tensor.matmul(
                                            ps[:],
                                            w_sel[:, t * P:(t + 1) * P],
                                            x_sel[src][:, c0:c0 + cw],
                                            start=(i_mm == 0),
                                            stop=(i_mm == n_mm - 1),
                                        )
                                        i_mm += 1
                                start = 2 * coff + half
                                nc.scalar.copy(
                                    o_t[:, start:start + 2 * cw - 1:2], ps[:]
                                )
                            coff += cw
                        nc.sync.dma_start(
                            out=o[r0:r0 + P, 2 * p0:2 * p0 + 2 * pw], in_=o_t[:]
                        )
                        p0 += pw

                    # ---------------- DVE region ----------------
                    q0 = 0
                    for dw in dve_widths:
                        tw = dw + HALO
                        dv_t = dvein_pool.tile([P, tw], f32, tag="dv")
                        av_t = dvein_pool.tile([P, tw], f32, tag="av")
                        nc.scalar.dma_start(out=dv_t[:], in_=dv[r0:r0 + P, q0:q0 + tw])
                        nc.scalar.dma_start(out=av_t[:], in_=av[r0:r0 + P, q0:q0 + tw])
                        srcs = (dv_t, av_t)

                        o_t = dveout_pool.tile([P, 2 * dw], f32, tag="dvo")
                        for half in (0, 1):
                            taps = list(range(8 * half, 8 * half + 8))
                            acc = acc_pool.tile([P, dw], f32, tag="acc")
                            src, j = _tap_src_shift(taps[0])
                            nc.vector.tensor_scalar(
                                acc[:],
                                srcs[src][:, j:j + dw],
                                c_sb[:, taps[0]:taps[0] + 1],
                                None,
                                Alu.mult,
                            )
                            for t in taps[1:-1]:
                                src, j = _tap_src_shift(t)
                                nc.vector.scalar_tensor_tensor(
                                    acc[:],
                                    srcs[src][:, j:j + dw],
                                    c_sb[:, t:t + 1],
                                    acc[:],
                                    Alu.mult,
                                    Alu.add,
                                )
                            t = taps[-1]
                            src, j = _tap_src_shift(t)
                            nc.vector.scalar_tensor_tensor(
                                o_t[:, half:half + 2 * dw - 1:2],
                                srcs[src][:, j:j + dw],
                                c_sb[:, t:t + 1],
                                acc[:],
                                Alu.mult,
                                Alu.add,
                            )
                        oc0 = 2 * (PE_U + q0)
                        nc.scalar.dma_start(
                            out=o[r0:r0 + P, oc0:oc0 + 2 * dw], in_=o_t[:]
                        )
                        q0 += dw
    nc.compile()
    return nc


def _prep_inputs(details, approximation, scaling, scaling_rec):
    d = np.ascontiguousarray(np.asarray(details, dtype=np.float32))
    a = np.ascontiguousarray(np.asarray(approximation, dtype=np.float32))
    s = np.asarray(scaling, dtype=np.float64)
    sr = np.asarray(scaling_rec, dtype=np.float64)

    g = sr[::-1].copy()
    g[1::2] *= -1.0
    coefs = np.concatenate([g[0::2], s[0::2], g[1::2], s[1::2]]).astype(np.float32)
    c_hi = coefs.astype(np.float16)
    c_lo = (coefs - c_hi.astype(np.float32)).astype(np.float16)

    eye16 = np.eye(P, dtype=np.float16)
    wh_np = np.zeros((P, 16 * P), np.float16)
    wl_np = np.zeros((P, 16 * P), np.float16)
    for t in range(16):
        wh_np[:, t * P:(t + 1) * P] = c_hi[t] * eye16
        wl_np[:, t * P:(t + 1) * P] = c_lo[t] * eye16
    c_np = np.tile(coefs[None, :], (P, 1)).astype(np.float32)

    # PE region slices (with halo) as fp16 hi/lo
    def split16(x):
        hi = x.astype(np.float16)
        lo = (x - hi.astype(np.float32)).astype(np.float16)
        return hi, lo

    d_pe = d[:, :PE_U + HALO]
    a_pe = a[:, :PE_U + HALO]
    dh_np, dl_np = split16(d_pe)
    ah_np, al_np = split16(a_pe)

    # DVE region slices (with circular halo) as fp32
    dv_np = np.ascontiguousarray(np.concatenate([d[:, PE_U:], d[:, :HALO]], axis=1))
    av_np = np.ascontiguousarray(np.concatenate([a[:, PE_U:], a[:, :HALO]], axis=1))

    return dh_np, dl_np, ah_np, al_np, dv_np, av_np, wh_np, wl_np, c_np


def make_in_maps(details, approximation, scaling, scaling_rec):
    dh_np, dl_np, ah_np, al_np, dv_np, av_np, wh_np, wl_np, c_np = _prep_inputs(
        details, approximation, scaling, scaling_rec
    )
    in_maps = []
    for core in range(N_CORES):
        r0 = core * ROWS_PER_CORE
        r1 = r0 + ROWS_PER_CORE
        in_maps.append(
            {
                "dh": dh_np[r0:r1],
                "dl": dl_np[r0:r1],
                "ah": ah_np[r0:r1],
                "al": al_np[r0:r1],
                "dv": dv_np[r0:r1],
                "av": av_np[r0:r1],
                "wh": wh_np,
                "wl": wl_np,
                "cc": c_np,
            }
        )
    return in_maps


def kernel(details, approximation, scaling, scaling_rec):
    if "nc" not in _CACHE:
        _CACHE["nc"] = _build()
    nc = _CACHE["nc"]

    from concourse.bass_utils import run_bass_kernel_spmd

    in_maps = make_in_maps(details, approximation, scaling, scaling_rec)
    res = run_bass_kernel_spmd(nc, in_maps, core_ids=list(range(N_CORES)))
    return np.concatenate([r["o"] for r in res.results], axis=0)


# revision 23
# speedup vs baseline: 1.0589x; 1.0589x over previous
"""Trainium2 Bass kernel for nn_BackwardTransformLayer (inverse wavelet step).

Math (polyphase form of the reference):
    g = flip(scaling_rec); g[1::2] *= -1
    out[i, 2u]   = sum_{j=0..3} g[2j]   * d[i, (u+j)   % M] + s[2j]   * a[i, (u+j)   % M]
    out[i, 2u+1] = sum_{j=0..3} g[2j+1] * d[i, (u+1+j) % M] + s[2j+1] * a[i, (u+1+j) % M]

i.e. two 4-tap circular FIRs along the free dim per output polyphase, summed
(16 MACs per input column).  Shifts are free (SBUF column-offset views).

Engine split (per core, 512 rows):
  - PE region (u in [0, PE_U)): taps as scaled-identity matmuls accumulating
    in PSUM.  fp32 matmul is 4 cyc/row on TRN2, so inputs are split host-side
    into fp16 hi + fp16 lo (same total bytes as fp32) and each tap runs as
    3 full-rate fp16 matmuls: c_hi*x_hi + c_hi*x_lo + c_lo*x_hi  (the dropped
    c_lo*x_lo term is ~2^-22 relative).  ScalarE drains PSUM into the output
    tile with a stride-2 write that interleaves even/odd.
  - DVE region (u in [PE_U, M)): fp32 scalar_tensor_tensor FMA chains; the
    last tap writes interleaved (stride-2) directly.

Sharding: embarrassingly parallel over rows; 512 rows per core x 8 cores.
"""

import numpy as np

P = 128                        # SBUF partitions
M = 8192                       # input columns
N_ROWS = 4096
N_CORES = 8
ROWS_PER_CORE = N_ROWS // N_CORES    # 512
OUT_M = 2 * M
HALO = 4                       # max shift reach (odd half uses j+1 <= 4)
NBLK = ROWS_PER_CORE // P      # 4 row blocks per core

PE_U = 3648                    # columns computed on PE (fp16 x3 path)
DVE_U = M - PE_U               # columns computed on DVE (fp32 path) = 4544
PE_PAN = 1824                  # PE panel width (2 panels)
DVE_PAN = 2272                 # DVE panel width (2 panels)
_CACHE = {}


def _tap_src_shift(t):
    """Tap table: t=0..7 feed the even polyphase, 8..15 the odd one."""
    src = 0 if (t % 8) < 4 else 1        # 0 -> details, 1 -> approximation
    shift = t % 4 + (1 if t >= 8 else 0)
    return src, shift


def _build(reps=1):
    import contextlib

    import concourse.bacc as bacc
    import concourse.mybir as mybir
    from concourse.tile import TileContext

    f32 = mybir.dt.float32
    f16 = mybir.dt.float16
    Alu = mybir.AluOpType

    nc = bacc.Bacc("TRN2", target_bir_lowering=False, debug=False)
    dh = nc.dram_tensor("dh", [ROWS_PER_CORE, PE_U + HALO], f16, kind="ExternalInput")
    dl = nc.dram_tensor("dl", [ROWS_PER_CORE, PE_U + HALO], f16, kind="ExternalInput")
    ah = nc.dram_tensor("ah", [ROWS_PER_CORE, PE_U + HALO], f16, kind="ExternalInput")
    al = nc.dram_tensor("al", [ROWS_PER_CORE, PE_U + HALO], f16, kind="ExternalInput")
    dv = nc.dram_tensor("dv", [ROWS_PER_CORE, DVE_U + HALO], f32, kind="ExternalInput")
    av = nc.dram_tensor("av", [ROWS_PER_CORE, DVE_U + HALO], f32, kind="ExternalInput")
    wh = nc.dram_tensor("wh", [P, 16 * P], f16, kind="ExternalInput")
    wl = nc.dram_tensor("wl", [P, 16 * P], f16, kind="ExternalInput")
    cc = nc.dram_tensor("cc", [P, 16], f32, kind="ExternalInput")
    o = nc.dram_tensor("o", [ROWS_PER_CORE, OUT_M], f32, kind="ExternalOutput")

    with TileContext(nc) as tc:
        with (
            tc.tile_pool(name="const", bufs=1) as const_pool,
            tc.tile_pool(name="pein", bufs=3) as pein_pool,
            tc.tile_pool(name="peout", bufs=2) as peout_pool,
            tc.tile_pool(name="dvein", bufs=2) as dvein_pool,
            tc.tile_pool(name="dveout", bufs=2) as dveout_pool,
            tc.tile_pool(name="acc", bufs=2) as acc_pool,
            tc.tile_pool(name="psum", bufs=8, space="PSUM") as psum_pool,
        ):
            wh_sb = const_pool.tile([P, 16 * P], f16)
            nc.sync.dma_start(out=wh_sb[:], in_=wh[:])
            c_sb = const_pool.tile([P, 16], f32)
            nc.scalar.dma_start(out=c_sb[:], in_=cc[:])
            # wl is first needed ~16 matmuls into the first PSUM group
            # (pass-major order) — defer its DMA behind the first panel's
            # hi tiles so it doesn't delay the first matmul.
            wl_sb = const_pool.tile([P, 16 * P], f16)
            wl_loaded = [False]

            rep_ctx = tc.For_i(0, reps, 1) if reps > 1 else contextlib.nullcontext()
            with rep_ctx:
                for blk in range(NBLK):
                    r0 = blk * P

                    # First block's leading panels are halved so the first
                    # compute op starts after less DMA (shorter pipeline fill);
                    # last block's trailing panels are halved so the final
                    # stores and tail drain start earlier.
                    if blk == 0:
                        pe_widths = [PE_PAN // 4, PE_PAN // 4, PE_PAN // 2, PE_PAN]
                        dve_widths = [DVE_PAN // 4, DVE_PAN // 4, DVE_PAN // 2, DVE_PAN]
                    elif blk == NBLK - 1:
                        pe_widths = [PE_PAN, PE_PAN // 2, PE_PAN // 2]
                        dve_widths = [DVE_PAN, DVE_PAN // 2, DVE_PAN // 2]
                    else:
                        pe_widths = [PE_PAN, PE_PAN]
                        dve_widths = [DVE_PAN, DVE_PAN]

                    # ---------------- PE region ----------------
                    p0 = 0
                    for pw in pe_widths:
                        tw = pw + HALO
                        dh_t = pein_pool.tile([P, tw], f16, tag="dh")
                        dl_t = pein_pool.tile([P, tw], f16, tag="dl")
                        ah_t = pein_pool.tile([P, tw], f16, tag="ah")
                        al_t = pein_pool.tile([P, tw], f16, tag="al")
                        for src_dram, t_sb in ((dh, dh_t), (ah, ah_t), (dl, dl_t), (al, al_t)):
                            nc.sync.dma_start(
                                out=t_sb[:], in_=src_dram[r0:r0 + P, p0:p0 + tw]
                            )
                        if not wl_loaded[0]:
                            nc.sync.dma_start(out=wl_sb[:], in_=wl[:])
                            wl_loaded[0] = True
                        hi = (dh_t, ah_t)
                        lo = (dl_t, al_t)

                        chunks = []
                        rem = pw
                        while rem > 0:
                            c = min(512, rem)
                            chunks.append(c)
                            rem -= c

                        o_t = peout_pool.tile([P, 2 * pw], f32, tag="peo")
                        coff = 0
                        for cw in chunks:
                            for half in (0, 1):
                                ps = psum_pool.tile([P, cw], f32, tag="ps")
                                taps = list(range(8 * half, 8 * half + 8))
                                n_mm = 3 * len(taps)
                                i_mm = 0
                                for w_sel, x_sel in ((wh_sb, hi), (wh_sb, lo), (wl_sb, hi)):
                                    for t in taps:
                                        src, j = _tap_src_shift(t)
                                        c0 = coff + j
                                        nc.tensor.matmul(
                                            ps[:],
                                            w_sel[:, t * P:(t + 1) * P],
                                            x_sel[src][:, c0:c0 + cw],
                                            start=(i_mm == 0),
                                            stop=(i_mm == n_mm - 1),
                                        )
                                        i_mm += 1
                                start = 2 * coff + half
                                nc.scalar.copy(
                                    o_t[:, start:start + 2 * cw - 1:2], ps[:]
                                )
                            coff += cw
                        nc.sync.dma_start(
                            out=o[r0:r0 + P, 2 * p0:2 * p0 + 2 * pw], in_=o_t[:]
                        )
                        p0 += pw

                    # ---------------- DVE region ----------------
                    q0 = 0
                    for dw in dve_widths:
                        tw = dw + HALO
                        dv_t = dvein_pool.tile([P, tw], f32, tag="dv")
                        av_t = dvein_pool.tile([P, tw], f32, tag="av")
                        nc.scalar.dma_start(out=dv_t[:], in_=dv[r0:r0 + P, q0:q0 + tw])
                        nc.scalar.dma_start(out=av_t[:], in_=av[r0:r0 + P, q0:q0 + tw])
                        srcs = (dv_t, av_t)

                        o_t = dveout_pool.tile([P, 2 * dw], f32, tag="dvo")
                        for half in (0, 1):
                            taps = list(range(8 * half, 8 * half + 8))
                            acc = acc_pool.tile([P, dw], f32, tag="acc")
                            src, j = _tap_src_shift(taps[0])
                            nc.vector.tensor_scalar(
                                acc[:],
                                srcs[src][:, j:j + dw],
                                c_sb[:, taps[0]:taps[0] + 1],
                                None,
                                Alu.mult,
                            )
                            for t in taps[1:-1]:
                                src, j = _tap_src_shift(t)
                                nc.vector.scalar_tensor_tensor(
                                    acc[:],
                                    srcs[src][:, j:j + dw],
                                    c_sb[:, t:t + 1],
                                    acc[:],
                                    Alu.mult,
                                    Alu.add,
                                )
                            t = taps[-1]
                            src, j = _tap_src_shift(t)
                            nc.vector.scalar_tensor_tensor(
                                o_t[:, half:half + 2 * dw - 1:2],
                                srcs[src][:, j:j + dw],
                                c_sb[:, t:t + 1],
                                acc[:],
                                Alu.mult,
                                Alu.add,
                            )
                        oc0 = 2 * (PE_U + q0)
                        nc.scalar.dma_start(
                            out=o[r0:r0 + P, oc0:oc0 + 2 * dw], in_=o_t[:]
                        )
                        q0 += dw
    nc.compile()
    return nc


def _prep_inputs(details, approximation, scaling, scaling_rec):
    d = np.ascontiguousarray(np.asarray(details, dtype=np.float32))
    a = np.ascontiguousarray(np.asarray(approximation, dtype=np.float32))
    s = np.asarray(scaling, dtype=np.float64)
    sr = np.asarray(scaling_rec, dtype=np.float64)

    g = sr[::-1].copy()
    g[1::2] *= -1.0
    coefs = np.concatenate([g[0::2], s[0::2], g[1::2], s[1::2]]).astype(np.float32)
    c_hi = coefs.astype(np.float16)
    c_lo = (coefs - c_hi.astype(np.float32)).astype(np.float16)

    eye16 = np.eye(P, dtype=np.float16)
    wh_np = np.zeros((P, 16 * P), np.float16)
    wl_np = np.zeros((P, 16 * P), np.float16)
    for t in range(16):
        wh_np[:, t * P:(t + 1) * P] = c_hi[t] * eye16
        wl_np[:, t * P:(t + 1) * P] = c_lo[t] * eye16
    c_np = np.tile(coefs[None, :], (P, 1)).astype(np.float32)

    # PE region slices (with halo) as fp16 hi/lo
    def split16(x):
        hi = x.astype(np.float16)
        lo = (x - hi.astype(np.float32)).astype(np.float16)
        return hi, lo

    d_pe = d[:, :PE_U + HALO]
    a_pe = a[:, :PE_U + HALO]
    dh_np, dl_np = split16(d_pe)
    ah_np, al_np = split16(a_pe)

    # DVE region slices (with circular halo) as fp32
    dv_np = np.ascontiguousarray(np.concatenate([d[:, PE_U:], d[:, :HALO]], axis=1))
    av_np = np.ascontiguousarray(np.concatenate([a[:, PE_U:], a[:, :HALO]], axis=1))

    return dh_np, dl_np, ah_np, al_np, dv_np, av_np, wh_np, wl_np, c_np


def make_in_maps(details, approximation, scaling, scaling_rec):
    dh_np, dl_np, ah_np, al_np, dv_np, av_np, wh_np, wl_np, c_np = _prep_inputs(
        details, approximation, scaling, scaling_rec
    )
    in_maps = []
    for core in range(N_CORES):
        r0 = core * ROWS_PER_CORE
        r1 = r0 + ROWS_PER_CORE
        in_maps.append(
            {
                "dh": dh_np[r0:r1],
                "dl": dl_np[r0:r1],
                "ah": ah_np[r0:r1],
                "al": al_np[r0:r1],
                "dv": dv_np[r0:r1],
                "av": av_np[r0:r1],
                "wh": wh_np,
                "wl": wl_np,
                "cc": c_np,
            }
        )
    return in_maps


def kernel(details, approximation, scaling, scaling_rec):
    if "nc" not in _CACHE:
        _CACHE["nc"] = _build()
    nc = _CACHE["nc"]

    from concourse.bass_utils import run_bass_kernel_spmd

    in_maps = make_in_maps(details, approximation, scaling, scaling_rec)
    res = run_bass_kernel_spmd(nc, in_maps, core_ids=list(range(N_CORES)))
    return np.concatenate([r["o"] for r in res.results], axis=0)
